# revision 29
# baseline (speedup 1.0000x reference)
"""Single-head causal attention (B=16, T=1024, C=768, H=64) on 8 TRN2 cores.

Strategy: data-parallel over batch (2 batch elements per core), weights
replicated, all matmuls bf16 (fp32 PE runs at 1/4 rate; rel-err budget 2e-2).
Per batch element, on-device:
  qT[h,t], kT[h,t], vT[h,t] = W.T @ x.T     (x.T supplied by host, bf16)
  v transposed to natural [s, h] layout via one DMA-xbar transpose into an
  augmented [v | 1 | 0] stationary tile (ones column -> softmax denominator).
  Per 128-row key block i (unified over the full 1024 query cols):
    S.T[s, t] = kT_i.T @ qT          (valid cols [128i : 1024] only)
    E = exp(scale * S.T)             (one ACT op per block, no wasted cols)
    diagonal block masked with an upper-triangular multiply (GpSimd)
    out_aug.T[:, t] += va_i.T @ E    (accumulated per 512-col PSUM bank)
  The unnormalized [65, 512] results (numerator rows 0..63, denominator row
  64) are copied to SBUF and DMA'd out; the host does the final divide and
  [h,t] -> [t,h] transpose (1M flops, negligible).
"""

import numpy as np
from contextlib import ExitStack

import concourse.bass as bass
import concourse.tile as tile
from concourse import mybir
from concourse.vector_clock import ScopedClock
from concourse.masks import make_upper_triangular

f32 = mybir.dt.float32
bf16 = mybir.dt.bfloat16
AF = mybir.ActivationFunctionType

B, T, C, H = 16, 1024, 768, 64
NCORES = 8
BPC = B // NCORES          # batches per core = 2
CT = C // 128              # 6 contraction chunks
TT = T // 128              # 8 t/s blocks of 128
NJ = T // 512              # 2 psum banks of 512 query cols
SCALE = 1.0 / np.sqrt(H).astype(np.float32)


def _patched_drain_and_barrier(self, tick_clock, wait_clock):
    # This container's walrus build allows only ONE sync-wait command on a
    # CTRL-class (Drain) instruction; stock Tile attaches one wait per live
    # semaphore to a single tail drain. Split into a chain of drains.
    nc = self.nc
    drain_inst = nc.sync.drain()
    wait_clock.add_sem_waits(
        drain_inst.ins, ScopedClock({None: tick_clock.global_clock})
    )
    mi = drain_inst.ins
    si = mi.sync_info
    if si is not None and len(si.on_wait) > 1:
        waits = list(si.on_wait)
        mi.sync_info = mybir.SyncInfo(on_wait=waits[:1], on_update=list(si.on_update))
        for w in waits[1:]:
            d2 = nc.sync.drain()
            d2.ins.sync_info = mybir.SyncInfo(on_wait=[w], on_update=[])
    nc.all_engine_barrier()
    assert self.sems is not None
    popped = nc._tile_sem_poison_stack.pop()
    assert popped is self._sem_poison
    nc.clear_and_free_semaphores(list(self.sems.allocated().values()))
    nc.all_engine_barrier()


tile.TileContext._drain_and_barrier = _patched_drain_and_barrier


def _split_excess_waits(nc, max_waits=1):
    # Same walrus limitation for every instruction class: at most one
    # sync-wait command. Hoist extra waits onto standalone EventSemaphore
    # instructions placed immediately before, on the same engine.
    n_new = 0
    for f in nc.m.functions:
        for bb in f.blocks:
            new_insts = []
            for inst in bb.instructions:
                si = inst.sync_info
                if si is not None and len(si.on_wait) > max_waits:
                    waits = list(si.on_wait)
                    for k, w in enumerate(waits[max_waits:]):
                        ev = mybir.InstEventSemaphore(
                            name=f"{inst.name}-xw{k}", ins=[], outs=[]
                        )
                        ev.engine = inst.engine
                        ev.sync_info = mybir.SyncInfo(on_wait=[w], on_update=[])
                        new_insts.append(ev)
                        n_new += 1
                    inst.sync_info = mybir.SyncInfo(
                        on_wait=waits[:max_waits], on_update=list(si.on_update)
                    )
                new_insts.append(inst)
            bb.instructions = new_insts
    return n_new


def _build_nc():
    nc = bass.Bass()
    # x.T packed per batch as [128, CT*T]: col c*T+t holds x[b, t, 128*c+p]
    xt = nc.declare_dram_parameter("xt", [BPC, 128, CT * T], bf16, isOutput=False)
    # col-packed: [wq (CT*H) | wkv (CT*128)]
    wd = nc.declare_dram_parameter("w", [128, CT * H + CT * 128], bf16, isOutput=False)
    # col 0 = bq (rows 0..63, rest zero), col 1 = [bk | bv]
    bd = nc.declare_dram_parameter("bias", [128, 2], f32, isOutput=False)
    # unnormalized transposed output: rows 0..63 numerator, row 64 denominator
    out = nc.declare_dram_parameter("out", [BPC, NJ, 65, 512], f32, isOutput=True)

    with ExitStack() as ctx:
        tc = ctx.enter_context(tile.TileContext(nc))
        const = ctx.enter_context(tc.tile_pool(name="const", bufs=1))
        xt_pool = ctx.enter_context(tc.tile_pool(name="xt_pool", bufs=2))
        proj = ctx.enter_context(tc.tile_pool(name="proj", bufs=2))
        vaug_pool = ctx.enter_context(tc.tile_pool(name="vaug_pool", bufs=2))
        et_pool = ctx.enter_context(tc.tile_pool(name="et_pool", bufs=6))
        avs_pool = ctx.enter_context(tc.tile_pool(name="avs_pool", bufs=4))
        ps = ctx.enter_context(tc.tile_pool(name="ps", bufs=1, space="PSUM"))

        tri = const.tile([128, 128], bf16)
        make_upper_triangular(nc, tri, val=1.0, diag=True)

        # First x chunk ahead of everything; weights split so wq (needed by
        # the very first matmul) lands before wkv.
        xt_sbs = []
        for b in range(BPC):
            xt_sbs.append(
                xt_pool.tile([128, CT * T], bf16, name=f"xt_{b}", tag="xt")
            )
        # weights + bias go out on the ACT HWDGE ring so their descriptor
        # generation overlaps the x-chunk DMAs on the SP ring
        w_sb = const.tile([128, CT * H + CT * 128], bf16)
        nc.scalar.dma_start(w_sb[:, 0 : CT * H], wd[:, 0 : CT * H])
        nc.sync.dma_start(xt_sbs[0][:, 0:T], xt[0, :, 0:T])
        nc.scalar.dma_start(w_sb[:, CT * H :], wd[:, CT * H :])
        wq_sb = w_sb[:, 0 : CT * H]
        wkv_sb = w_sb[:, CT * H :]
        b_sb = const.tile([128, 2], f32)
        nc.scalar.dma_start(b_sb[:, :], bd[:, :])
        bq_sb = b_sb[0:H, 0:1]
        bkv_sb = b_sb[:, 1:2]

        qTs, kv_sbs, va_alls = [], [], []
        for b in range(BPC):
            # b=0: fine-grained head chunks so the first proj matmul starts
            # after ~256KB; b=1 is prefetched during b=0 compute, one DMA.
            xt_sb = xt_sbs[b]
            chunks = [(1, 2), (2, CT)] if b == 0 else [(0, CT)]
            for c0, c1 in chunks:
                nc.sync.dma_start(
                    xt_sb[:, T * c0 : T * c1], xt[b, :, T * c0 : T * c1]
                )

            qT = proj.tile([H, T], bf16, name=f"qT_{b}", tag="qT")
            # k rows 0..63, v rows 64..127 (one PSUM->SBUF copy for both)
            kv_sb = proj.tile([128, T], bf16, name=f"kv_{b}", tag="kv")

            for n in range(T // 512):
                ncol = slice(512 * n, 512 * (n + 1))
                pq = ps.tile([128, 512], f32, name=f"pq_{b}_{n}", tag="st", bufs=6)
                for c in range(CT):
                    nc.tensor.matmul(
                        pq[:H, :],
                        lhsT=wq_sb[:, H * c : H * (c + 1)],
                        rhs=xt_sb[:, T * c + 512 * n : T * c + 512 * (n + 1)],
                        start=(c == 0),
                        stop=(c == CT - 1),
                    )
                nc.vector.tensor_scalar_add(qT[:, ncol], pq[:H, :], bq_sb[:, :])
                pkv = ps.tile([128, 512], f32, name=f"pkv_{b}_{n}", tag="st", bufs=6)
                for c in range(CT):
                    nc.tensor.matmul(
                        pkv[:, :],
                        lhsT=wkv_sb[:, 128 * c : 128 * (c + 1)],
                        rhs=xt_sb[:, T * c + 512 * n : T * c + 512 * (n + 1)],
                        start=(c == 0),
                        stop=(c == CT - 1),
                    )
                nc.vector.tensor_scalar_add(
                    kv_sb[:, ncol], pkv[:, :], bkv_sb[:, :]
                )

            # va_all block i = [v_i | 1 | 0] (128 cols each): one DMA-xbar
            # transpose fills all v columns; ones/zeros are memset once.
            va_all = vaug_pool.tile([128, TT * 128], bf16, name=f"va_{b}", tag="vaug")
            va3 = va_all.rearrange("p (i c) -> p i c", c=128)
            nc.gpsimd.memset(va3[:, :, H : H + 1], 1.0)
            nc.gpsimd.memset(va3[:, :, H + 1 : 128], 0.0)
            nc.scalar.dma_start_transpose(va3[:, :, 0:H], kv_sb[H:128, :])
            qTs.append(qT)
            kv_sbs.append(kv_sb)
            va_alls.append(va_all)

        for b in range(BPC):
            qT, kv_sb, va_all = qTs[b], kv_sbs[b], va_alls[b]
            op = [
                ps.tile([128, 512], f32, name=f"op_{b}_{j}", tag="ps_out", bufs=2)
                for j in range(NJ)
            ]
            for i in range(TT):
                g0 = 128 * i
                et = et_pool.tile([128, 1024], bf16, name=f"et_{b}_{i}", tag="et")
                if i < 4:
                    st0 = ps.tile([128, 512], f32, name=f"sta_{b}_{i}", tag="st", bufs=6)
                    nc.tensor.matmul(
                        st0[:, g0:512],
                        lhsT=kv_sb[:H, g0 : g0 + 128],
                        rhs=qT[:, g0:512],
                        start=True,
                        stop=True,
                    )
                    nc.scalar.activation(
                        et[:, g0:512], st0[:, g0:512], AF.Exp, scale=float(SCALE)
                    )
                    st1 = ps.tile([128, 512], f32, name=f"stb_{b}_{i}", tag="st", bufs=6)
                    nc.tensor.matmul(
                        st1[:, :],
                        lhsT=kv_sb[:H, g0 : g0 + 128],
                        rhs=qT[:, 512:1024],
                        start=True,
                        stop=True,
                    )
                    nc.scalar.activation(
                        et[:, 512:1024], st1[:, :], AF.Exp, scale=float(SCALE)
                    )
                else:
                    st0 = ps.tile([128, 512], f32, name=f"sta_{b}_{i}", tag="st", bufs=6)
                    nc.tensor.matmul(
                        st0[:, g0 - 512 : 512],
                        lhsT=kv_sb[:H, g0 : g0 + 128],
                        rhs=qT[:, g0:1024],
                        start=True,
                        stop=True,
                    )
                    nc.scalar.activation(
                        et[:, g0:1024], st0[:, g0 - 512 : 512], AF.Exp,
                        scale=float(SCALE),
                    )
                nc.vector.tensor_mul(
                    et[:, g0 : g0 + 128], et[:, g0 : g0 + 128], tri[:, :]
                )
                if i < 4:
                    nc.tensor.matmul(
                        op[0][:, g0:512],
                        lhsT=va_all[:, g0 : g0 + 128],
                        rhs=et[:, g0:512],
                        start=(i == 0),
                        stop=(i == 3),
                    )
                g1 = max(512, g0)
                nc.tensor.matmul(
                    op[1][:, g1 - 512 : 512],
                    lhsT=va_all[:, g0 : g0 + 128],
                    rhs=et[:, g1:1024],
                    start=(i == 0),
                    stop=(i == TT - 1),
                )

            for j in range(NJ):
                av = avs_pool.tile([65, 512], f32, name=f"av_{b}_{j}", tag="av")
                if b == BPC - 1 and j == NJ - 1:
                    # tail: cols [0:384] complete one AV-matmul earlier than
                    # [384:512]; split so the bulk ships before the last
                    # block, on both HWDGE rings for parallel generation
                    nc.vector.tensor_copy(av[:, 0:384], op[j][0:65, 0:384])
                    nc.scalar.dma_start(out[b, j, :, 0:384], av[:, 0:384])
                    nc.vector.tensor_copy(av[:, 384:512], op[j][0:65, 384:512])
                    nc.sync.dma_start(out[b, j, :, 384:512], av[:, 384:512])
                else:
                    nc.vector.tensor_copy(av[:, :], op[j][0:65, :])
                    nc.sync.dma_start(out[b, j], av[:, :])
    _split_excess_waits(nc)
    return nc


_NC_CACHE = None


def _get_nc():
    global _NC_CACHE
    if _NC_CACHE is None:
        _NC_CACHE = _build_nc()
    return _NC_CACHE


_BF16_NP = mybir.dt.np(bf16)


def _prep_in_maps(x, Wq, bq, Wk, bk, Wv, bv):
    x = np.asarray(x, dtype=np.float32)
    Wq = np.asarray(Wq, dtype=np.float32)
    Wk = np.asarray(Wk, dtype=np.float32)
    Wv = np.asarray(Wv, dtype=np.float32)
    bq = np.asarray(bq, dtype=np.float32)
    bk = np.asarray(bk, dtype=np.float32)
    bv = np.asarray(bv, dtype=np.float32)

    wq_p = Wq.reshape(CT, 128, H).transpose(1, 0, 2).reshape(128, CT * H)
    wkv = np.concatenate([Wk, Wv], axis=1)  # [C, 128]
    wkv_p = wkv.reshape(CT, 128, 128).transpose(1, 0, 2).reshape(128, CT * 128)
    w_p = np.ascontiguousarray(
        np.concatenate([wq_p, wkv_p], axis=1)
    ).astype(_BF16_NP)
    b_p = np.zeros((128, 2), np.float32)
    b_p[:H, 0] = bq
    b_p[:H, 1] = bk
    b_p[H:, 1] = bv

    # x [B, T, C] -> per-core [BPC, 128, CT*T] bf16 where [p, c*T+t] = x[b,t,128c+p]
    xb = x.astype(_BF16_NP)
    in_maps = []
    for i in range(NCORES):
        xs = (
            xb[BPC * i : BPC * (i + 1)]
            .reshape(BPC, T, CT, 128)
            .transpose(0, 3, 2, 1)
            .reshape(BPC, 128, CT * T)
        )
        in_maps.append({"xt": np.ascontiguousarray(xs), "w": w_p, "bias": b_p})
    return in_maps


def _postprocess(raw):
    """raw [B, NJ, 65, 512] -> [B, T, H]: divide by denominator row, transpose."""
    num = raw[:, :, :H, :]                  # [B, NJ, H, 512]
    den = raw[:, :, H : H + 1, :]           # [B, NJ, 1, 512]
    o = num / den
    return np.ascontiguousarray(o.transpose(0, 1, 3, 2).reshape(-1, T, H))


def run(inputs, trace=False, **spmd_kwargs):
    from concourse.bass_utils import run_bass_kernel_spmd

    nc = _get_nc()
    in_maps = _prep_in_maps(**inputs)
    res = run_bass_kernel_spmd(
        nc, in_maps, list(range(NCORES)), trace=trace, **spmd_kwargs
    )
    raw = np.concatenate([res.results[i]["out"] for i in range(NCORES)], axis=0)
    return _postprocess(raw).astype(np.float32, copy=False), res


def kernel(**inputs) -> np.ndarray:
    out, _ = run(inputs)
    return out
